# revision 9
# baseline (speedup 1.0000x reference)
"""Trainium2 Bass kernel for nn_ChaoticDecoder.

Math notes (exact algebraic simplifications of the reference):
  - alpha = softmax_seq(cat([x, states_b]) @ Wa + ba). The states term and ba
    are constant along the seq axis, so they cancel inside the softmax:
    alpha = softmax_seq(x @ Wa[:D]).  Hence alpha and
    context = sum_s alpha * x are step-invariant -> computed once.
  - The per-step work is just two LSTM cells with the constant input
    `context`:  g_t = (ctx @ Wi + b) + h_t @ Wh, with the first part (gx)
    precomputed.

Sharding: data-parallel over batch, 8 cores x 16 batch each. No collectives.

On-chip layout: everything transposed. States h,c live as [128 (hid), 2, 16]
(cells f/v side by side, batch on the free dim), gates as [128 (gate), 8, 16]
with slot order  i_f, i_v, f_f, f_v, o_f, o_v, g_f, g_v  so that sigmoid runs
on one contiguous [128,96] slab and tanh on [128,32].
"""

import numpy as np

BS, SEQ, D, H, OUT = 128, 64, 64, 128, 4
NCORES = 8
BPC = BS // NCORES  # batch per core = 16

_CACHE = {}


def _build(n_steps=SEQ):
    import concourse.bass as bass
    import concourse.mybir as mybir
    import concourse.tile as tile
    from concourse import bacc
    from concourse.masks import make_identity

    fp32 = mybir.dt.float32
    nc = bacc.Bacc("TRN2", target_bir_lowering=False)

    # ---- I/O ----
    x_d = nc.dram_tensor("x", [BPC, SEQ, D], fp32, kind="ExternalInput")
    wa_d = nc.dram_tensor("Wa", [D + 4 * H, D], fp32, kind="ExternalInput")
    wi_d = nc.dram_tensor("Wi", [D, 4 * H], fp32, kind="ExternalInput")
    wh_d = nc.dram_tensor("Wh", [H, 4 * H], fp32, kind="ExternalInput")
    b_d = nc.dram_tensor("b", [4 * H], fp32, kind="ExternalInput")
    wvi_d = nc.dram_tensor("Wvi", [D, 4 * H], fp32, kind="ExternalInput")
    wvh_d = nc.dram_tensor("Wvh", [H, 4 * H], fp32, kind="ExternalInput")
    bv_d = nc.dram_tensor("bv", [4 * H], fp32, kind="ExternalInput")
    wfc_d = nc.dram_tensor("Wfc", [2 * H, OUT], fp32, kind="ExternalInput")
    bfc_d = nc.dram_tensor("bfc", [OUT], fp32, kind="ExternalInput")
    out_d = nc.dram_tensor("out", [BPC, OUT], fp32, kind="ExternalOutput")

    # slot order: i_f,i_v,g_f,g_v,f_f,f_v,o_f,o_v ; gate j blocks in W are i,f,g,o
    SLOT = {("f", 0): 0, ("v", 0): 1, ("f", 2): 2, ("v", 2): 3,
            ("f", 1): 4, ("v", 1): 5, ("f", 3): 6, ("v", 3): 7}

    with tile.TileContext(nc) as tc:
        with (
            tc.tile_pool(name="const", bufs=1) as const,
            tc.tile_pool(name="pre", bufs=1) as pre,
            tc.tile_pool(name="ps_tp", bufs=1, space="PSUM") as ps_tp,
            tc.tile_pool(name="ps_once", bufs=1, space="PSUM") as ps_once,
            tc.tile_pool(name="gpsum", bufs=2, space="PSUM") as gpsum,
            tc.tile_pool(name="work", bufs=3) as work,
            tc.tile_pool(name="state", bufs=3) as state,
        ):
            # ---- constant loads ----
            wa1 = const.tile([D, D], fp32, tag="wa1")  # Wa[:D] as lhsT [d_in, d_out]
            nc.sync.dma_start(out=wa1, in_=wa_d[0:D, :])
            wi_sb = const.tile([D, 4 * H], fp32, tag="wi")
            nc.sync.dma_start(out=wi_sb, in_=wi_d[:, :])
            wvi_sb = const.tile([D, 4 * H], fp32, tag="wvi")
            nc.sync.dma_start(out=wvi_sb, in_=wvi_d[:, :])
            wh_sb = const.tile([H, 4 * H], fp32, tag="wh")
            nc.sync.dma_start(out=wh_sb, in_=wh_d[:, :])
            wvh_sb = const.tile([H, 4 * H], fp32, tag="wvh")
            nc.sync.dma_start(out=wvh_sb, in_=wvh_d[:, :])
            # biases as [128, 4] (partition = gate dim within block, free = j)
            bT = const.tile([H, 4], fp32, tag="bT")
            nc.sync.dma_start(out=bT, in_=b_d.rearrange("(j p) -> p j", p=H))
            bvT = const.tile([H, 4], fp32, tag="bvT")
            nc.sync.dma_start(out=bvT, in_=bv_d.rearrange("(j p) -> p j", p=H))
            wfc_sb = const.tile([H, 2, OUT], fp32, tag="wfc")
            nc.sync.dma_start(out=wfc_sb, in_=wfc_d.rearrange("(n p) o -> p n o", p=H))
            bfc_sb = const.tile([BPC, OUT], fp32, tag="bfc")
            nc.sync.dma_start(
                out=bfc_sb,
                in_=bass.AP(tensor=bfc_d, offset=0, ap=[[0, BPC], [1, OUT]]),
            )
            nc.vector.tensor_scalar_mul(
                out=wh_sb[:, 2 * H:3 * H], in0=wh_sb[:, 2 * H:3 * H], scalar1=2.0)
            nc.vector.tensor_scalar_mul(
                out=wvh_sb[:, 2 * H:3 * H], in0=wvh_sb[:, 2 * H:3 * H], scalar1=2.0)
            ident = const.tile([128, 128], fp32, tag="ident")
            make_identity(nc, ident)

            # One-time 1x1 "toucher" matmuls: advance PE's observed vector
            # clock past each DMA-queue semaphore so real matmuls later carry
            # at most ONE sync wait (walrus limit on the LDWEIGHTS struct).
            scratch = ps_tp.tile([1, 16], fp32, tag="tp")
            touch_list = [ident, wa1, wi_sb, wvi_sb, wh_sb, wvh_sb, wfc_sb]
            for k, w in enumerate(touch_list):
                lhs1 = w[0:1, 0:1] if len(w.shape) == 2 else w[0:1, 0:1, 0:1]
                nc.tensor.matmul(
                    scratch[0:1, k:k + 1],
                    lhs1.rearrange("a b -> a (b)") if len(w.shape) == 2 else
                    lhs1.rearrange("a b c -> a (b c)"),
                    ident[0:1, 0:1], start=True, stop=True)

            # ---- x load + transpose to [d, (b s)] ----
            x_nat = pre.tile([128, 8, D], fp32, tag="xnat")  # rows (b*64+s) tiled by 128
            nc.sync.dma_start(
                out=x_nat, in_=x_d.rearrange("b s d -> (b s) d").rearrange(
                    "(n p) d -> p n d", p=128)
            )
            xT = pre.tile([D, 8, 128], fp32, tag="xT")  # [d, b(2/tile) s]
            for i in range(8):
                tp = ps_tp.tile([D, 128], fp32, tag="tp")
                nc.tensor.transpose(tp, x_nat[:, i, :], ident)
                nc.vector.tensor_copy(out=xT[:, i, :], in_=tp)

            # ---- attention (once): xa = x @ Wa1 ; softmax over s ; context ----
            xa_ps = ps_once.tile([D, 2, 512], fp32, tag="xa")
            xTf = xT.rearrange("p n q -> p (n q)")
            for hhalf in range(2):
                nc.tensor.matmul(
                    xa_ps[:, hhalf, :], wa1, xTf[:, hhalf * 512:(hhalf + 1) * 512],
                    start=True, stop=True,
                )
            e_sb = pre.tile([D, BPC, SEQ], fp32, tag="e")
            nc.scalar.activation(
                out=e_sb.rearrange("p a b -> p (a b)"),
                in_=xa_ps.rearrange("p a b -> p (a b)"),
                func=mybir.ActivationFunctionType.Exp,
            )
            den = work.tile([D, BPC], fp32, tag="den")
            nc.vector.reduce_sum(out=den, in_=e_sb, axis=mybir.AxisListType.X)
            rden = work.tile([D, BPC], fp32, tag="rden")
            nc.vector.reciprocal(out=rden, in_=den)
            wgt = pre.tile([D, BPC, SEQ], fp32, tag="wgt")
            nc.vector.tensor_mul(
                out=wgt.rearrange("p a b -> p (a b)"),
                in0=e_sb.rearrange("p a b -> p (a b)"),
                in1=xT.rearrange("p n q -> p (n q)"),
            )
            num = work.tile([D, BPC], fp32, tag="num")
            nc.vector.reduce_sum(out=num, in_=wgt, axis=mybir.AxisListType.X)
            ctx = pre.tile([D, BPC], fp32, tag="ctx")
            nc.vector.tensor_mul(out=ctx, in0=num, in1=rden)

            # ---- gx = ctx @ Wi + b (transposed, slot-ordered) ----
            gx_ps = ps_once.tile([H, 8, BPC], fp32, tag="gxps")
            for j in range(4):
                nc.tensor.matmul(
                    gx_ps[:, SLOT[("f", j)], :], wi_sb[:, j * H:(j + 1) * H], ctx,
                    start=True, stop=True)
                nc.tensor.matmul(
                    gx_ps[:, SLOT[("v", j)], :], wvi_sb[:, j * H:(j + 1) * H], ctx,
                    start=True, stop=True)
            gx = pre.tile([H, 8, BPC], fp32, tag="gx")
            for j in range(4):
                nc.vector.tensor_scalar_add(
                    out=gx[:, SLOT[("f", j)], :], in0=gx_ps[:, SLOT[("f", j)], :],
                    scalar1=bT[:, j:j + 1])
                nc.vector.tensor_scalar_add(
                    out=gx[:, SLOT[("v", j)], :], in0=gx_ps[:, SLOT[("v", j)], :],
                    scalar1=bvT[:, j:j + 1])

            nc.vector.tensor_scalar_mul(
                out=gx[:, 2:4, :].rearrange("p a b -> p (a b)"),
                in0=gx[:, 2:4, :].rearrange("p a b -> p (a b)"), scalar1=2.0)
            gxT_ps = ps_tp.tile([128, 128], fp32, tag="tp")
            nc.tensor.transpose(gxT_ps, gx.rearrange("p a b -> p (a b)"), ident)
            gxT = pre.tile([128, 128], fp32, tag="gxT")
            nc.vector.tensor_copy(out=gxT, in_=gxT_ps)

            # ---- init states to 0 ----
            h_cur = state.tile([H, 2, BPC], fp32, tag="h")
            nc.vector.memset(h_cur, 0.0)
            c_cur = state.tile([H, 2, BPC], fp32, tag="c")
            nc.vector.memset(c_cur, 0.0)

            # ---- the 64-step recurrence ----
            # Two PSUM tiles per step in different banks so the f/o matmuls
            # never share a bank with the slots sigma_a is reading.  Each
            # starts pre-loaded with its half of gx via one matmul against
            # identity; the Wh matmuls accumulate on top (start=False).
            JA = {("f", 0): 0, ("f", 2): 2, ("v", 0): 1, ("v", 2): 3}
            JB = {("f", 1): 0, ("f", 3): 2, ("v", 1): 1, ("v", 3): 3}

            def remat(which):
                pgx = gpsum.tile([H, 4, BPC], fp32, tag=f"pg{which}")
                lo = 0 if which == "a" else 64
                nc.tensor.matmul(pgx.rearrange("p a b -> p (a b)"), gxT,
                                 ident[:, lo:lo + 64], start=True, stop=False,
                                 skip_group_check=True)
                return pgx

            pga_cur = remat("a")
            pgb_cur = remat("b")
            for t in range(n_steps):
                for (cell, j), sl in JA.items():
                    nc.tensor.matmul(
                        pga_cur[:, sl, :], (wh_sb if cell == "f" else wvh_sb)[:, j * H:(j + 1) * H],
                        h_cur[:, 0 if cell == "f" else 1, :], start=False,
                        stop=True, skip_group_check=True)
                for (cell, j), sl in JB.items():
                    nc.tensor.matmul(
                        pgb_cur[:, sl, :], (wh_sb if cell == "f" else wvh_sb)[:, j * H:(j + 1) * H],
                        h_cur[:, 0 if cell == "f" else 1, :], start=False,
                        stop=True, skip_group_check=True)
                pga_next = remat("a") if t < n_steps - 1 else None
                pgb_next = remat("b") if t < n_steps - 1 else None
                gs_a = work.tile([H, 4, BPC], fp32, tag="gsa")  # sig(i,i,2g,2g)
                nc.scalar.activation(
                    out=gs_a.rearrange("p a b -> p (a b)"),
                    in_=pga_cur.rearrange("p a b -> p (a b)"),
                    func=mybir.ActivationFunctionType.Sigmoid)
                gs_b = work.tile([H, 4, BPC], fp32, tag="gsb")  # sig(f,f,o,o)
                nc.scalar.activation(
                    out=gs_b.rearrange("p a b -> p (a b)"),
                    in_=pgb_cur.rearrange("p a b -> p (a b)"),
                    func=mybir.ActivationFunctionType.Sigmoid)
                tg = work.tile([H, 2, BPC], fp32, tag="tg")  # tanh(g)=2*sig(2g)-1
                nc.vector.tensor_scalar(
                    out=tg.rearrange("p a b -> p (a b)"),
                    in0=gs_a[:, 2:4, :].rearrange("p a b -> p (a b)"),
                    scalar1=2.0, scalar2=1.0,
                    op0=mybir.AluOpType.mult, op1=mybir.AluOpType.subtract)
                t2 = work.tile([H, 2, BPC], fp32, tag="t2")
                nc.vector.tensor_mul(
                    out=t2.rearrange("p a b -> p (a b)"),
                    in0=gs_a[:, 0:2, :].rearrange("p a b -> p (a b)"),
                    in1=tg.rearrange("p a b -> p (a b)"))
                t1 = work.tile([H, 2, BPC], fp32, tag="t1")
                nc.vector.tensor_mul(
                    out=t1.rearrange("p a b -> p (a b)"),
                    in0=gs_b[:, 0:2, :].rearrange("p a b -> p (a b)"),
                    in1=c_cur.rearrange("p a b -> p (a b)"))
                c_new = state.tile([H, 2, BPC], fp32, tag="c")
                nc.vector.tensor_add(
                    out=c_new.rearrange("p a b -> p (a b)"),
                    in0=t1.rearrange("p a b -> p (a b)"),
                    in1=t2.rearrange("p a b -> p (a b)"))
                tc_t = work.tile([H, 2, BPC], fp32, tag="tc")
                nc.scalar.activation(
                    out=tc_t.rearrange("p a b -> p (a b)"),
                    in_=c_new.rearrange("p a b -> p (a b)"),
                    func=mybir.ActivationFunctionType.Tanh)
                h_new = state.tile([H, 2, BPC], fp32, tag="h")
                nc.vector.tensor_mul(
                    out=h_new[:, 0, :], in0=gs_b[:, 2, :], in1=tc_t[:, 0, :])
                nc.vector.tensor_mul(
                    out=h_new[:, 1, :], in0=gs_b[:, 3, :], in1=tc_t[:, 1, :])
                h_cur, c_cur = h_new, c_new
                pga_cur, pgb_cur = pga_next, pgb_next

            # ---- head: out = [h_f | h_v] @ Wfc + bfc ----
            o_ps = ps_tp.tile([BPC, OUT], fp32, tag="tp")
            nc.tensor.matmul(o_ps, h_cur[:, 0, :], wfc_sb[:, 0, :],
                             start=True, stop=False)
            nc.tensor.matmul(o_ps, h_cur[:, 1, :], wfc_sb[:, 1, :],
                             start=False, stop=True)
            o_sb = work.tile([BPC, OUT], fp32, tag="osb")
            nc.vector.tensor_add(out=o_sb, in0=o_ps, in1=bfc_sb)
            nc.sync.dma_start(out=out_d[:, :], in_=o_sb)

    nc.compile()
    return nc


def kernel(**inputs):
    from concourse import bass_utils

    if "nc" not in _CACHE:
        _CACHE["nc"] = _build()
    nc = _CACHE["nc"]

    x = np.ascontiguousarray(inputs["x"], dtype=np.float32)
    shared = {
        k: np.ascontiguousarray(inputs[k], dtype=np.float32)
        for k in ["Wa", "Wi", "Wh", "b", "Wvi", "Wvh", "bv", "Wfc", "bfc"]
    }
    in_maps = []
    for c in range(NCORES):
        m = dict(shared)
        m["x"] = x[c * BPC:(c + 1) * BPC]
        in_maps.append(m)

    res = bass_utils.run_bass_kernel_spmd(nc, in_maps, core_ids=list(range(NCORES)))
    out = np.concatenate([r["out"] for r in res.results], axis=0)
    return out.astype(np.float32)


# revision 10
# speedup vs baseline: 1.0040x; 1.0040x over previous
"""Trainium2 Bass kernel for nn_ChaoticDecoder.

Math notes (exact algebraic simplifications of the reference):
  - alpha = softmax_seq(cat([x, states_b]) @ Wa + ba). The states term and ba
    are constant along the seq axis, so they cancel inside the softmax:
    alpha = softmax_seq(x @ Wa[:D]).  Hence alpha and
    context = sum_s alpha * x are step-invariant -> computed once.
  - The per-step work is just two LSTM cells with the constant input
    `context`:  g_t = (ctx @ Wi + b) + h_t @ Wh, with the first part (gx)
    precomputed.

Sharding: data-parallel over batch, 8 cores x 16 batch each. No collectives.

On-chip layout: everything transposed. States h,c live as [128 (hid), 2, 16]
(cells f/v side by side, batch on the free dim), gates as [128 (gate), 8, 16]
with slot order  i_f, i_v, f_f, f_v, o_f, o_v, g_f, g_v  so that sigmoid runs
on one contiguous [128,96] slab and tanh on [128,32].
"""

import numpy as np

BS, SEQ, D, H, OUT = 128, 64, 64, 128, 4
NCORES = 8
BPC = BS // NCORES  # batch per core = 16

_CACHE = {}


def _build(n_steps=SEQ):
    import concourse.bass as bass
    import concourse.mybir as mybir
    import concourse.tile as tile
    from concourse import bacc
    from concourse.masks import make_identity

    fp32 = mybir.dt.float32
    nc = bacc.Bacc("TRN2", target_bir_lowering=False)

    # ---- I/O ----
    x_d = nc.dram_tensor("x", [BPC, SEQ, D], fp32, kind="ExternalInput")
    wa_d = nc.dram_tensor("Wa", [D + 4 * H, D], fp32, kind="ExternalInput")
    wi_d = nc.dram_tensor("Wi", [D, 4 * H], fp32, kind="ExternalInput")
    wh_d = nc.dram_tensor("Wh", [H, 4 * H], fp32, kind="ExternalInput")
    b_d = nc.dram_tensor("b", [4 * H], fp32, kind="ExternalInput")
    wvi_d = nc.dram_tensor("Wvi", [D, 4 * H], fp32, kind="ExternalInput")
    wvh_d = nc.dram_tensor("Wvh", [H, 4 * H], fp32, kind="ExternalInput")
    bv_d = nc.dram_tensor("bv", [4 * H], fp32, kind="ExternalInput")
    wfc_d = nc.dram_tensor("Wfc", [2 * H, OUT], fp32, kind="ExternalInput")
    bfc_d = nc.dram_tensor("bfc", [OUT], fp32, kind="ExternalInput")
    out_d = nc.dram_tensor("out", [BPC, OUT], fp32, kind="ExternalOutput")

    # slot order: i_f,i_v,g_f,g_v,f_f,f_v,o_f,o_v ; gate j blocks in W are i,f,g,o
    SLOT = {("f", 0): 0, ("v", 0): 1, ("f", 2): 2, ("v", 2): 3,
            ("f", 1): 4, ("v", 1): 5, ("f", 3): 6, ("v", 3): 7}

    with tile.TileContext(nc) as tc:
        with (
            tc.tile_pool(name="const", bufs=1) as const,
            tc.tile_pool(name="pre", bufs=1) as pre,
            tc.tile_pool(name="ps_tp", bufs=1, space="PSUM") as ps_tp,
            tc.tile_pool(name="ps_once", bufs=1, space="PSUM") as ps_once,
            tc.tile_pool(name="gpsum", bufs=2, space="PSUM") as gpsum,
            tc.tile_pool(name="work", bufs=3) as work,
            tc.tile_pool(name="state", bufs=3) as state,
        ):
            # ---- constant loads ----
            wa1 = const.tile([D, D], fp32, tag="wa1")  # Wa[:D] as lhsT [d_in, d_out]
            nc.sync.dma_start(out=wa1, in_=wa_d[0:D, :])
            wi_sb = const.tile([D, 4 * H], fp32, tag="wi")
            nc.sync.dma_start(out=wi_sb, in_=wi_d[:, :])
            wvi_sb = const.tile([D, 4 * H], fp32, tag="wvi")
            nc.sync.dma_start(out=wvi_sb, in_=wvi_d[:, :])
            wh_sb = const.tile([H, 4 * H], fp32, tag="wh")
            nc.sync.dma_start(out=wh_sb, in_=wh_d[:, :])
            wvh_sb = const.tile([H, 4 * H], fp32, tag="wvh")
            nc.sync.dma_start(out=wvh_sb, in_=wvh_d[:, :])
            # biases as [128, 4] (partition = gate dim within block, free = j)
            bT = const.tile([H, 4], fp32, tag="bT")
            nc.sync.dma_start(out=bT, in_=b_d.rearrange("(j p) -> p j", p=H))
            bvT = const.tile([H, 4], fp32, tag="bvT")
            nc.sync.dma_start(out=bvT, in_=bv_d.rearrange("(j p) -> p j", p=H))
            wfc_sb = const.tile([H, 2, OUT], fp32, tag="wfc")
            nc.sync.dma_start(out=wfc_sb, in_=wfc_d.rearrange("(n p) o -> p n o", p=H))
            bfc_sb = const.tile([BPC, OUT], fp32, tag="bfc")
            nc.sync.dma_start(
                out=bfc_sb,
                in_=bass.AP(tensor=bfc_d, offset=0, ap=[[0, BPC], [1, OUT]]),
            )
            nc.vector.tensor_scalar_mul(
                out=wh_sb[:, 2 * H:3 * H], in0=wh_sb[:, 2 * H:3 * H], scalar1=2.0)
            nc.vector.tensor_scalar_mul(
                out=wvh_sb[:, 2 * H:3 * H], in0=wvh_sb[:, 2 * H:3 * H], scalar1=2.0)
            ident = const.tile([128, 128], fp32, tag="ident")
            make_identity(nc, ident)

            # One-time 1x1 "toucher" matmuls: advance PE's observed vector
            # clock past each DMA-queue semaphore so real matmuls later carry
            # at most ONE sync wait (walrus limit on the LDWEIGHTS struct).
            scratch = ps_tp.tile([1, 16], fp32, tag="tp")
            touch_list = [ident, wa1, wi_sb, wvi_sb, wh_sb, wvh_sb, wfc_sb]
            for k, w in enumerate(touch_list):
                lhs1 = w[0:1, 0:1] if len(w.shape) == 2 else w[0:1, 0:1, 0:1]
                nc.tensor.matmul(
                    scratch[0:1, k:k + 1],
                    lhs1.rearrange("a b -> a (b)") if len(w.shape) == 2 else
                    lhs1.rearrange("a b c -> a (b c)"),
                    ident[0:1, 0:1], start=True, stop=True)

            # ---- x load + transpose to [d, (b s)] ----
            x_nat = pre.tile([128, 8, D], fp32, tag="xnat")  # rows (b*64+s) tiled by 128
            nc.sync.dma_start(
                out=x_nat, in_=x_d.rearrange("b s d -> (b s) d").rearrange(
                    "(n p) d -> p n d", p=128)
            )
            xT = pre.tile([D, 8, 128], fp32, tag="xT")  # [d, b(2/tile) s]
            for i in range(8):
                tp = ps_tp.tile([D, 128], fp32, tag="tp")
                nc.tensor.transpose(tp, x_nat[:, i, :], ident)
                nc.vector.tensor_copy(out=xT[:, i, :], in_=tp)

            # ---- attention (once): xa = x @ Wa1 ; softmax over s ; context ----
            xa_ps = ps_once.tile([D, 2, 512], fp32, tag="xa")
            xTf = xT.rearrange("p n q -> p (n q)")
            for hhalf in range(2):
                nc.tensor.matmul(
                    xa_ps[:, hhalf, :], wa1, xTf[:, hhalf * 512:(hhalf + 1) * 512],
                    start=True, stop=True,
                )
            e_sb = pre.tile([D, BPC, SEQ], fp32, tag="e")
            nc.scalar.activation(
                out=e_sb.rearrange("p a b -> p (a b)"),
                in_=xa_ps.rearrange("p a b -> p (a b)"),
                func=mybir.ActivationFunctionType.Exp,
            )
            den = work.tile([D, BPC], fp32, tag="den")
            nc.vector.reduce_sum(out=den, in_=e_sb, axis=mybir.AxisListType.X)
            rden = work.tile([D, BPC], fp32, tag="rden")
            nc.vector.reciprocal(out=rden, in_=den)
            wgt = pre.tile([D, BPC, SEQ], fp32, tag="wgt")
            nc.vector.tensor_mul(
                out=wgt.rearrange("p a b -> p (a b)"),
                in0=e_sb.rearrange("p a b -> p (a b)"),
                in1=xT.rearrange("p n q -> p (n q)"),
            )
            num = work.tile([D, BPC], fp32, tag="num")
            nc.vector.reduce_sum(out=num, in_=wgt, axis=mybir.AxisListType.X)
            ctx = pre.tile([D, BPC], fp32, tag="ctx")
            nc.vector.tensor_mul(out=ctx, in0=num, in1=rden)

            # ---- gx = ctx @ Wi + b (transposed, slot-ordered) ----
            gx_ps = ps_once.tile([H, 8, BPC], fp32, tag="gxps")
            for j in range(4):
                nc.tensor.matmul(
                    gx_ps[:, SLOT[("f", j)], :], wi_sb[:, j * H:(j + 1) * H], ctx,
                    start=True, stop=True)
                nc.tensor.matmul(
                    gx_ps[:, SLOT[("v", j)], :], wvi_sb[:, j * H:(j + 1) * H], ctx,
                    start=True, stop=True)
            gx = pre.tile([H, 8, BPC], fp32, tag="gx")
            for j in range(4):
                nc.vector.tensor_scalar_add(
                    out=gx[:, SLOT[("f", j)], :], in0=gx_ps[:, SLOT[("f", j)], :],
                    scalar1=bT[:, j:j + 1])
                nc.vector.tensor_scalar_add(
                    out=gx[:, SLOT[("v", j)], :], in0=gx_ps[:, SLOT[("v", j)], :],
                    scalar1=bvT[:, j:j + 1])

            nc.vector.tensor_scalar_mul(
                out=gx[:, 2:4, :].rearrange("p a b -> p (a b)"),
                in0=gx[:, 2:4, :].rearrange("p a b -> p (a b)"), scalar1=2.0)
            gxT_ps = ps_tp.tile([128, 128], fp32, tag="tp")
            nc.tensor.transpose(gxT_ps, gx.rearrange("p a b -> p (a b)"), ident)
            gxT = pre.tile([128, 128], fp32, tag="gxT")
            nc.vector.tensor_copy(out=gxT, in_=gxT_ps)

            # ---- init states to 0 ----
            h_cur = state.tile([H, 2, BPC], fp32, tag="h")
            nc.vector.memset(h_cur, 0.0)
            c_cur = state.tile([H, 2, BPC], fp32, tag="c")
            nc.vector.memset(c_cur, 0.0)

            # ---- the 64-step recurrence ----
            # Two PSUM tiles per step in different banks so the f/o matmuls
            # never share a bank with the slots sigma_a is reading.  Each
            # starts pre-loaded with its half of gx via one matmul against
            # identity; the Wh matmuls accumulate on top (start=False).
            JA = {("f", 0): 0, ("v", 0): 1, ("f", 2): 2, ("v", 2): 3}
            JB = {("f", 1): 0, ("v", 1): 1, ("f", 3): 2, ("v", 3): 3}

            def remat(which):
                pgx = gpsum.tile([H, 4, BPC], fp32, tag=f"pg{which}")
                lo = 0 if which == "a" else 64
                nc.tensor.matmul(pgx.rearrange("p a b -> p (a b)"), gxT,
                                 ident[:, lo:lo + 64], start=True, stop=False,
                                 skip_group_check=True)
                return pgx

            pga_cur = remat("a")
            pgb_cur = remat("b")
            for t in range(n_steps):
                for (cell, j), sl in JA.items():
                    nc.tensor.matmul(
                        pga_cur[:, sl, :], (wh_sb if cell == "f" else wvh_sb)[:, j * H:(j + 1) * H],
                        h_cur[:, 0 if cell == "f" else 1, :], start=False,
                        stop=True, skip_group_check=True)
                for (cell, j), sl in JB.items():
                    nc.tensor.matmul(
                        pgb_cur[:, sl, :], (wh_sb if cell == "f" else wvh_sb)[:, j * H:(j + 1) * H],
                        h_cur[:, 0 if cell == "f" else 1, :], start=False,
                        stop=True, skip_group_check=True)
                pga_next = remat("a") if t < n_steps - 1 else None
                pgb_next = remat("b") if t < n_steps - 1 else None
                gs_a = work.tile([H, 4, BPC], fp32, tag="gsa")  # sig(i,i,2g,2g)
                nc.scalar.activation(
                    out=gs_a.rearrange("p a b -> p (a b)"),
                    in_=pga_cur.rearrange("p a b -> p (a b)"),
                    func=mybir.ActivationFunctionType.Sigmoid)
                gs_b = work.tile([H, 4, BPC], fp32, tag="gsb")  # sig(f,f,o,o)
                nc.scalar.activation(
                    out=gs_b.rearrange("p a b -> p (a b)"),
                    in_=pgb_cur.rearrange("p a b -> p (a b)"),
                    func=mybir.ActivationFunctionType.Sigmoid)
                tg = work.tile([H, 2, BPC], fp32, tag="tg")  # tanh(g)=2*sig(2g)-1
                nc.vector.tensor_scalar(
                    out=tg.rearrange("p a b -> p (a b)"),
                    in0=gs_a[:, 2:4, :].rearrange("p a b -> p (a b)"),
                    scalar1=2.0, scalar2=1.0,
                    op0=mybir.AluOpType.mult, op1=mybir.AluOpType.subtract)
                t2 = work.tile([H, 2, BPC], fp32, tag="t2")
                nc.vector.tensor_mul(
                    out=t2.rearrange("p a b -> p (a b)"),
                    in0=gs_a[:, 0:2, :].rearrange("p a b -> p (a b)"),
                    in1=tg.rearrange("p a b -> p (a b)"))
                t1 = work.tile([H, 2, BPC], fp32, tag="t1")
                nc.vector.tensor_mul(
                    out=t1.rearrange("p a b -> p (a b)"),
                    in0=gs_b[:, 0:2, :].rearrange("p a b -> p (a b)"),
                    in1=c_cur.rearrange("p a b -> p (a b)"))
                c_new = state.tile([H, 2, BPC], fp32, tag="c")
                nc.vector.tensor_add(
                    out=c_new.rearrange("p a b -> p (a b)"),
                    in0=t1.rearrange("p a b -> p (a b)"),
                    in1=t2.rearrange("p a b -> p (a b)"))
                tc_t = work.tile([H, 2, BPC], fp32, tag="tc")
                nc.scalar.activation(
                    out=tc_t.rearrange("p a b -> p (a b)"),
                    in_=c_new.rearrange("p a b -> p (a b)"),
                    func=mybir.ActivationFunctionType.Tanh)
                h_new = state.tile([H, 2, BPC], fp32, tag="h")
                nc.vector.tensor_mul(
                    out=h_new.rearrange("p a b -> p (a b)"),
                    in0=gs_b[:, 2:4, :].rearrange("p a b -> p (a b)"),
                    in1=tc_t.rearrange("p a b -> p (a b)"))
                h_cur, c_cur = h_new, c_new
                pga_cur, pgb_cur = pga_next, pgb_next

            # ---- head: out = [h_f | h_v] @ Wfc + bfc ----
            o_ps = ps_tp.tile([BPC, OUT], fp32, tag="tp")
            nc.tensor.matmul(o_ps, h_cur[:, 0, :], wfc_sb[:, 0, :],
                             start=True, stop=False)
            nc.tensor.matmul(o_ps, h_cur[:, 1, :], wfc_sb[:, 1, :],
                             start=False, stop=True)
            o_sb = work.tile([BPC, OUT], fp32, tag="osb")
            nc.vector.tensor_add(out=o_sb, in0=o_ps, in1=bfc_sb)
            nc.sync.dma_start(out=out_d[:, :], in_=o_sb)

    nc.compile()
    return nc


def kernel(**inputs):
    from concourse import bass_utils

    if "nc" not in _CACHE:
        _CACHE["nc"] = _build()
    nc = _CACHE["nc"]

    x = np.ascontiguousarray(inputs["x"], dtype=np.float32)
    shared = {
        k: np.ascontiguousarray(inputs[k], dtype=np.float32)
        for k in ["Wa", "Wi", "Wh", "b", "Wvi", "Wvh", "bv", "Wfc", "bfc"]
    }
    in_maps = []
    for c in range(NCORES):
        m = dict(shared)
        m["x"] = x[c * BPC:(c + 1) * BPC]
        in_maps.append(m)

    res = bass_utils.run_bass_kernel_spmd(nc, in_maps, core_ids=list(range(NCORES)))
    out = np.concatenate([r["out"] for r in res.results], axis=0)
    return out.astype(np.float32)
